# revision 2
# baseline (speedup 1.0000x reference)
# Trainium2 Bass kernel for nn_MultiCondLayer:
#   out[b,o,n] = (sum_k (cond[b] @ W[k].T)[o,n] + sum_k b[k,o]) * x_mask[b,0,n]
# Key algebraic reduction: sum_k Linear_k(x) == Linear(x) with W' = sum_k W[k],
# b' = sum_k b[k]  (4x FLOP reduction vs. the naive einsum over k).
#
# Sharding: data-parallel over batch B=8 across the 8 NeuronCores (one batch
# element per core); the reduced [1024,1024] weight is replicated.
#
# Numerics: x and W' are cast to bf16 on the host (measured end-to-end rel
# error ~2.4e-3 vs the 2e-2 gate; PE bf16 rate == fp32r rate, but input HBM
# traffic halves: 36 MB -> 26 MB per core). PSUM accumulates in fp32 and
# outputs store fp32.
#
# Per-core compute: [1024c,4096n] x [1024c,1024o] as 512 PE matmuls
# (128x128 bf16 lhsT, 128x512 bf16 rhs) accumulating in PSUM, evicted by a
# single fused DVE op: (psum+bias)*mask.
#
# Schedule: x-in streams alone on the Sync HWDGE queue in per-c [128,1024]
# chunks (2KB descriptors); weights (o-halved chunks), bias and out-stores
# ride the Activation HWDGE queue; the x_mask row is broadcast across
# partitions on-chip by the PE (ones outer product). Matmuls run
# c-outer/o4/nsub-inner so each weight tile feeds 2 back-to-back matmuls
# and 8 PSUM banks stay in flight; evictions+stores chase each psum group.

import numpy as np
import ml_dtypes

import concourse.bass as bass
import concourse.mybir as mybir
import concourse.tile as tile
from concourse import bacc
from concourse.bass_utils import run_bass_kernel_spmd

P = 128
B, C, N = 8, 1024, 4096
O = 1024
NT = 512                 # matmul free dim = one fp32 PSUM bank
CO, OO, NN = C // P, O // P, N // NT
F32 = mybir.dt.float32
F32R = mybir.dt.float32r
BF16 = mybir.dt.bfloat16

N_CORES = 8

NSUP = 1024              # n superchunk width (2 KB bf16 DMA descriptors)
NSUPS = N // NSUP        # 4
NSUB = NSUP // NT        # 2 psum-width subchunks per superchunk


def build_module():
    nc = bacc.Bacc("TRN2", target_bir_lowering=False, debug=False,
                   num_devices=N_CORES)
    x = nc.dram_tensor("x", [C, N], BF16, kind="ExternalInput")    # cond[b]
    wt = nc.dram_tensor("wt", [C, O], BF16, kind="ExternalInput")  # (sum_k W[k]).T
    # bias pre-transposed on host to [128, OO] so the DMA is 128 contiguous
    # 32B rows instead of 1024 4-byte gather descriptors.
    bv = nc.dram_tensor("bv", [P, OO], F32, kind="ExternalInput")
    mk = nc.dram_tensor("mk", [N], F32R, kind="ExternalInput")      # x_mask[b,0]
    out = nc.dram_tensor("out", [O, N], F32, kind="ExternalOutput")

    x_r = x.ap().rearrange("(c p) n -> p c n", p=P)      # [128, CO, N]
    wt_r = wt.ap().rearrange("(c p) o -> p c o", p=P)    # [128, CO, O]

    with tile.TileContext(nc) as tc:
        with (
            tc.tile_pool(name="consts", bufs=1) as consts,
            tc.tile_pool(name="xs", bufs=2) as xs,
            tc.tile_pool(name="outs", bufs=16) as outs,
            tc.tile_pool(name="ps", bufs=8, space="PSUM") as psp,
        ):
            # Mask broadcast built on-chip: the 16 KB mask row lands
            # instantly, then the (cold, otherwise DMA-starved) PE
            # outer-products it with a ones column into all 128 partitions.
            mkrow_sb = consts.tile([1, N], F32R)
            nc.scalar.dma_start(mkrow_sb[:], mk.ap()[None, :])
            ones_sb = consts.tile([1, P], F32)
            nc.vector.memset(ones_sb[:], 1.0)
            mask_sb = consts.tile([P, N], F32)
            for n in range(NN):
                # One full rotation of the shared 8-bank psum tag; the DVE
                # copies release the slots before the first real group lands.
                mps = psp.tile([P, NT], F32, name=f"mps_{n}", tag="ps")
                nc.tensor.matmul(mps[:], ones_sb[:].bitcast(F32R),
                                 mkrow_sb[:, n * NT:(n + 1) * NT],
                                 start=True, stop=True)
                nc.vector.tensor_copy(mask_sb[:, n * NT:(n + 1) * NT], mps[:])
            # Weights in per-(o-half, c) chunks: the first matmul is gated by
            # just w[og0,c0]+x[c0]. og0 weights are interleaved with the first
            # superchunk's x chunks; og1 weights follow.
            OH = O // 2
            w_sb = consts.tile([P, CO, O], BF16)
            bias_sb = consts.tile([P, OO], F32)
            for c in range(CO):
                nc.scalar.dma_start(w_sb[:, c, 0:OH], wt_r[:, c, 0:OH])
            nc.scalar.dma_start(bias_sb[:], bv.ap())
            for c in range(CO):
                nc.scalar.dma_start(w_sb[:, c, OH:O], wt_r[:, c, OH:O])

            for ns in range(NSUPS):
                x_sb = xs.tile([P, CO, NSUP], BF16, name=f"x_sb_{ns}",
                               tag="x_sb")
                for c in range(CO):
                    nc.sync.dma_start(
                        x_sb[:, c, :], x_r[:, c, ns * NSUP:(ns + 1) * NSUP])
                for og in range(2):
                    # 8 psum groups = 4 o-chunks x 2 n-subchunks; each weight
                    # tile feeds 2 back-to-back matmuls (nsub pair).
                    pss = [[psp.tile([P, NT], F32, name=f"ps_{ns}_{og}_{o4}_{nsub}",
                                     tag="ps")
                            for nsub in range(NSUB)] for o4 in range(4)]
                    for c in range(CO):
                        for o4 in range(4):
                            o = og * 4 + o4
                            for nsub in range(NSUB):
                                nc.tensor.matmul(
                                    pss[o4][nsub][:],
                                    w_sb[:, c, o * P:(o + 1) * P],
                                    x_sb[:, c, nsub * NT:(nsub + 1) * NT],
                                    start=(c == 0),
                                    stop=(c == CO - 1),
                                )
                    for o4 in range(4):
                        o = og * 4 + o4
                        for nsub in range(NSUB):
                            n0 = ns * NSUP + nsub * NT
                            ot = outs.tile([P, NT], F32,
                                           name=f"ot_{ns}_{og}_{o4}_{nsub}",
                                           tag="ot")
                            nc.vector.scalar_tensor_tensor(
                                ot[:], pss[o4][nsub][:],
                                bias_sb[:, o:o + 1], mask_sb[:, n0:n0 + NT],
                                op0=mybir.AluOpType.add, op1=mybir.AluOpType.mult,
                            )
                            nc.scalar.dma_start(
                                out.ap()[o * P:(o + 1) * P, n0:n0 + NT], ot[:])
    nc.compile()
    return nc


_NC_CACHE = None


def _get_module():
    global _NC_CACHE
    if _NC_CACHE is None:
        _NC_CACHE = build_module()
    return _NC_CACHE


def _make_in_maps(cond, x_mask, W, b):
    wt = np.ascontiguousarray(
        W.sum(axis=0).T.astype(ml_dtypes.bfloat16))                # [C, O] bf16
    bv = np.ascontiguousarray(
        b.sum(axis=0).reshape(OO, P).T, dtype=np.float32)          # [128, OO]
    in_maps = []
    for core in range(N_CORES):
        in_maps.append({
            "x": np.ascontiguousarray(
                np.asarray(cond[core]).astype(ml_dtypes.bfloat16)),
            "wt": wt,
            "bv": bv,
            "mk": np.ascontiguousarray(x_mask[core, 0], dtype=np.float32),
        })
    return in_maps


def run(cond, x_mask, W, b, trace=False, trace_cores=None):
    """Run on hardware; returns (out [B,O,N] fp32, BassKernelResults)."""
    nc = _get_module()
    in_maps = _make_in_maps(cond, x_mask, W, b)
    res = run_bass_kernel_spmd(
        nc, in_maps, core_ids=list(range(N_CORES)),
        trace=trace, trace_cores=trace_cores,
    )
    out = np.stack([res.results[i]["out"] for i in range(N_CORES)], axis=0)
    return out, res


def kernel(cond, x_mask, W, b):
    out, _ = run(cond, x_mask, W, b)
    return out
